# revision 4
# baseline (speedup 1.0000x reference)
"""Trainium2 Bass kernel for nn_AggrHGraphConvWindow (3x GraphConv -> LeakyReLU -> 2-layer LSTM).

v2: ACT-engine-paced design (sigmoid/tanh are ACT-only and dominate the
cost-model floor at ~650us).
- 9 skewed LSTM j-streams, each starting as soon as its 4 conv row-tiles
  spilled; unit = (j, t, layer); round-robin emission keeps every engine's
  in-order stream dependency-smooth.
- Layer-0 gates: ONE fused [x;h0] fp8 DoubleRow matmul pair (hi+lo weight
  compensation) per gate -> l0 PE cost 1/4 of bf16, so PE stays below ACT
  even at mid p-state (the cost model's ramp rule).
- Layer-1 stays bf16 (accuracy headroom).
- Prelu on DVE, cell adds on GPSIMD, all DMAs on the SP queue, gt at DVE 4x.
"""

import os
import numpy as np
import ml_dtypes

BF16 = np.float16  # fp16: same cost as bf16 on PE/DVE, 8x finer mantissa
FP8 = ml_dtypes.float8_e4m3

N_NODE, N_POD, N_SVC = 500, 30000, 3000
T, F, H = 16, 64, 128
NCORES = 8
P = 128

NODE_PC = 64
POD_PC = 3750
SVC_PC = 376

POD_TILES = (POD_PC + P - 1) // P   # 30
SVC_TILES = (SVC_PC + P - 1) // P   # 3
N_TILES = POD_TILES + SVC_TILES     # 33
NODE_ROW0 = POD_PC - (POD_TILES - 1) * P  # 38
R_CORE = N_TILES * P  # 4224

LSTM_TILES = [(j * 512, 512) for j in range(R_CORE // 512)]
if R_CORE % 512:
    LSTM_TILES.append((512 * (R_CORE // 512), R_CORE % 512))
NJ = len(LSTM_TILES)  # 9
KR = 4  # x/h0 ring depth per j-stream

_COMPILED = {}


def _degrees(src, dst, n_src, n_dst):
    dout = np.bincount(src, minlength=n_src).astype(np.float64)
    din = np.bincount(dst, minlength=n_dst).astype(np.float64)
    return (1.0 / np.sqrt(np.maximum(dout, 1.0)), 1.0 / np.sqrt(np.maximum(din, 1.0)))


def _prep(inputs):
    nf = np.asarray(inputs["node_feat"]).reshape(N_NODE, T * F)
    pf = np.asarray(inputs["pod_feat"]).reshape(N_POD, T * F)
    sf = np.asarray(inputs["svc_feat"]).reshape(N_SVC, T * F)

    in_src = np.asarray(inputs["inst_node_src"]).astype(np.int64)
    in_dst = np.asarray(inputs["inst_node_dst"]).astype(np.int64)
    ni_src = np.asarray(inputs["node_inst_src"]).astype(np.int64)
    ni_dst = np.asarray(inputs["node_inst_dst"]).astype(np.int64)
    sc_src = np.asarray(inputs["svc_call_src"]).astype(np.int64)
    sc_dst = np.asarray(inputs["svc_call_dst"]).astype(np.int64)

    ro_in, ri_in = _degrees(in_src, in_dst, N_POD, N_NODE)
    ro_ni, ri_ni = _degrees(ni_src, ni_dst, N_NODE, N_POD)
    ro_sc, ri_sc = _degrees(sc_src, sc_dst, N_SVC, N_SVC)

    def route(src, dst, w, kind):
        if kind == 0:
            core = dst // NODE_PC
            q = dst - core * NODE_PC
            tile = np.full_like(dst, POD_TILES - 1)
            row = NODE_ROW0 + q
        elif kind == 1:
            core = dst // POD_PC
            q = dst - core * POD_PC
            tile = q // P
            row = q % P
        else:
            core = dst // SVC_PC
            q = dst - core * SVC_PC
            tile = POD_TILES + q // P
            row = q % P
        return core, tile, row, src, w

    ew_in = (ro_in[in_src] * ri_in[in_dst]).astype(np.float32)
    ew_ni = (ro_ni[ni_src] * ri_ni[ni_dst]).astype(np.float32)
    ew_sc = (ro_sc[sc_src] * ri_sc[sc_dst]).astype(np.float32)

    routed = {
        0: route(in_src, in_dst, ew_in, 0),
        1: route(ni_src, ni_dst, ew_ni, 1),
        2: route(sc_src, sc_dst, ew_sc, 2),
    }

    allc, allt, allr, alls, allw, allk = [], [], [], [], [], []
    for kind in (0, 1, 2):
        core, tile, row, src, w = routed[kind]
        allc.append(core); allt.append(tile); allr.append(row)
        alls.append(src); allw.append(w)
        allk.append(np.full_like(src, kind))
    core = np.concatenate(allc); tile = np.concatenate(allt)
    row = np.concatenate(allr); src = np.concatenate(alls)
    w = np.concatenate(allw); kd = np.concatenate(allk)
    order = np.lexsort((row, tile, core))
    core, tile, row, src, w, kd = (a[order] for a in (core, tile, row, src, w, kd))
    buckets = [[(np.zeros(0, np.int64),) * 4 for _ in range(N_TILES)] for _ in range(NCORES)]
    key = core * N_TILES + tile
    uniq, starts = np.unique(key, return_index=True)
    starts = list(starts) + [len(key)]
    for ui, k in enumerate(uniq):
        c, t = int(k) // N_TILES, int(k) % N_TILES
        s, e = starts[ui], starts[ui + 1]
        buckets[c][t] = (src[s:e], row[s:e], w[s:e], kd[s:e])

    K = []
    for t in range(N_TILES):
        mx = 1
        for c in range(NCORES):
            mx = max(mx, (len(buckets[c][t][0]) + P - 1) // P)
        K.append(mx)
    base = np.concatenate([[0], np.cumsum(K)]).astype(np.int64)
    C_total = int(base[-1])

    srcfeat = {0: pf, 1: nf, 2: sf}

    in_maps = []
    for c in range(NCORES):
        edst = np.zeros((C_total, P), dtype=np.float32)
        ew = np.zeros((C_total, P), dtype=np.float32)
        srcdup = np.zeros((C_total * P, T * F), dtype=BF16)

        for t in range(N_TILES):
            src, row, w, kde = buckets[c][t]
            n = len(src)
            b0 = int(base[t]) * P
            for kk in (0, 1, 2):
                mkk = kde == kk
                if mkk.any():
                    srcdup[b0:b0 + n][mkk] = srcfeat[kk][src[mkk]].astype(BF16)
            edst.reshape(-1)[b0:b0 + n] = row
            ew.reshape(-1)[b0:b0 + n] = w

        m = {
            "srcdup": srcdup,
            "edst": np.ascontiguousarray(edst.T),
            "ew": np.ascontiguousarray(ew.T),
        }
        in_maps.append(m)

    def conv_w(Wname):
        W = np.asarray(inputs[Wname])
        wt = W.transpose(1, 0, 2).reshape(F, T * H)
        return np.vstack([wt, wt]).astype(BF16)

    def conv_b8(bname):
        b = np.asarray(inputs[bname]).reshape(T, H)
        hi = b.astype(FP8)
        lo = (b - hi.astype(np.float32)).astype(FP8)
        out = np.empty((T, 2, H), dtype=FP8)
        out[:, 0, :] = hi
        out[:, 1, :] = lo
        return out.reshape(1, T * 2 * H)

    def perm_ifog(W):
        # rows [i,f,g,o] -> [i,f,o,2g] (g doubled: tanh(g)=2*sigmoid(2g)-1)
        return np.concatenate([W[0:128], W[128:256], W[384:512], 2.0 * W[256:384]], axis=0)

    def lstm_w(Wname):
        Wp = perm_ifog(np.asarray(inputs[Wname]))
        return np.ascontiguousarray(Wp.T).astype(BF16)  # (in_dim, 512)

    def lstm_w0_fused():
        # layer-0 fused [x;h] fp8 DoubleRow weights, hi/lo compensated.
        # layout [k=128, (gate,4)(lvl,2)(pair,2)(m,128)] -> [128, 2048] fp8
        wih = perm_ifog(np.asarray(inputs["Wih0"]).astype(np.float64)).T  # (128,512)
        whh = perm_ifog(np.asarray(inputs["Whh0"]).astype(np.float64)).T
        out = np.zeros((P, 4, 2, 2, H), dtype=FP8)
        for g in range(4):
            for pair, Wsrc in ((0, wih), (1, whh)):
                Wg = Wsrc[:, g * H:(g + 1) * H]
                hi = Wg.astype(FP8)
                lo = (Wg - hi.astype(np.float64)).astype(FP8)
                out[:, g, 0, pair, :] = hi
                out[:, g, 1, pair, :] = lo
        return out.reshape(P, 2048)

    def lstm_b8(b1, b2):
        b = np.asarray(inputs[b1]) + np.asarray(inputs[b2])
        bp = np.concatenate([b[0:128], b[128:256], b[384:512], 2.0 * b[256:384]])
        bp = bp.reshape(4, H)
        hi = bp.astype(FP8)
        lo = (bp - hi.astype(np.float32)).astype(FP8)
        out = np.empty((4, 2, H), dtype=FP8)
        out[:, 0, :] = hi
        out[:, 1, :] = lo
        return out.reshape(1, 4 * 2 * H)

    shared = {
        "wt_node": conv_w("W_in"), "wt_pod": conv_w("W_ni"), "wt_svc": conv_w("W_svc"),
        "cb8_node": conv_b8("b_in"), "cb8_pod": conv_b8("b_ni"), "cb8_svc": conv_b8("b_svc"),
        "w08": lstm_w0_fused(),
        "wih1": lstm_w("Wih1"), "whh1": lstm_w("Whh1"),
        "bias8_0": lstm_b8("bih0", "bhh0"), "bias8_1": lstm_b8("bih1", "bhh1"),
        "ones8": np.ones((1, 1024), dtype=FP8),
        "iota": np.broadcast_to(np.arange(P, dtype=np.float16), (P, P)).copy(),
    }
    for m in in_maps:
        m.update(shared)

    meta = (C_total, tuple(K))
    return meta, in_maps


# ----------------------------------------------------------------------------
# Device program
# ----------------------------------------------------------------------------

def _build(meta):
    import concourse.bass as bass
    import concourse.tile as tile
    import concourse.mybir as mybir

    C_total, K = meta
    f32 = mybir.dt.float32
    bf16 = mybir.dt.float16
    fp16 = mybir.dt.float16
    fp8 = mybir.dt.float8e4
    AF = mybir.ActivationFunctionType
    ALU = mybir.AluOpType
    DR = mybir.MatmulPerfMode.DoubleRow

    import concourse.bacc as bacc
    nc = bacc.Bacc("TRN2", target_bir_lowering=False, debug=False, enable_asserts=False)

    srcdup_d = nc.dram_tensor("srcdup", [C_total * P, T * F], bf16, kind="ExternalInput")
    edst_d = nc.dram_tensor("edst", [P, C_total], f32, kind="ExternalInput")
    ew_d = nc.dram_tensor("ew", [P, C_total], f32, kind="ExternalInput")
    wt_d = {k: nc.dram_tensor(f"wt_{k}", [P, T * H], bf16, kind="ExternalInput")
            for k in ("node", "pod", "svc")}
    cb8_d = {k: nc.dram_tensor(f"cb8_{k}", [1, T * 2 * H], fp8, kind="ExternalInput")
             for k in ("node", "pod", "svc")}
    w08_d = nc.dram_tensor("w08", [P, 2048], fp8, kind="ExternalInput")
    wih1_d = nc.dram_tensor("wih1", [H, 512], bf16, kind="ExternalInput")
    whh1_d = nc.dram_tensor("whh1", [H, 512], bf16, kind="ExternalInput")
    bias8_d = [nc.dram_tensor(f"bias8_{l}", [1, 4 * 2 * H], fp8, kind="ExternalInput")
               for l in range(2)]
    ones8_d = nc.dram_tensor("ones8", [1, 1024], fp8, kind="ExternalInput")
    iota_d = nc.dram_tensor("iota", [P, P], bf16, kind="ExternalInput")
    out_d = nc.dram_tensor("out", [P, T * R_CORE], bf16, kind="ExternalOutput")

    segments = [[("pod", 0, P)] for _ in range(POD_TILES - 1)]
    segments.append([("pod", 0, NODE_ROW0), ("node", NODE_ROW0, P)])
    segments += [[("svc", 0, P)] for _ in range(SVC_TILES)]
    base = np.concatenate([[0], np.cumsum(K)]).astype(int)

    with tile.TileContext(nc) as tc:
        with tc.tile_pool(name="dram", bufs=NJ, space="DRAM") as dramp, \
             tc.tile_pool(name="const", bufs=1) as constp:
            # x0 spill per LSTM tile, fp8, layout [h, t*512 + r]
            x0p = [dramp.tile([P, T * 512], fp8, tag="x0p", name=f"x0p_{j}")
                   for j in range(NJ)]

            edst_sb = constp.tile([P, C_total], f32)
            ew_sb = constp.tile([P, C_total], f32)
            iota_sb = constp.tile([P, P], bf16)
            nc.sync.dma_start(edst_sb[:], edst_d.ap())
            nc.sync.dma_start(ew_sb[:], ew_d.ap())
            nc.sync.dma_start(iota_sb[:], iota_d.ap())
            wt_sb, cb8_sb = {}, {}
            for k in ("node", "pod", "svc"):
                wt_sb[k] = constp.tile([P, T * H], bf16, name=f"wt_{k}_sb")
                cb8_sb[k] = constp.tile([1, T * 2 * H], fp8, name=f"cb8_{k}_sb")
                nc.sync.dma_start(wt_sb[k][:], wt_d[k].ap())
                nc.sync.dma_start(cb8_sb[k][:], cb8_d[k].ap())
            w08_sb = constp.tile([P, 2048], fp8, name="w08_sb")
            wih1_sb = constp.tile([H, 512], bf16, name="wih1_sb")
            whh1_sb = constp.tile([H, 512], bf16, name="whh1_sb")
            nc.sync.dma_start(w08_sb[:], w08_d.ap())
            nc.sync.dma_start(wih1_sb[:], wih1_d.ap())
            nc.sync.dma_start(whh1_sb[:], whh1_d.ap())
            bias8_sb = []
            for l in range(2):
                bias8_sb.append(constp.tile([1, 4 * 2 * H], fp8, name=f"bias8_{l}_sb"))
                nc.sync.dma_start(bias8_sb[l][:], bias8_d[l].ap())
            ones8_sb = constp.tile([1, 1024], fp8)
            nc.sync.dma_start(ones8_sb[:], ones8_d.ap())

            srcdup_ap = srcdup_d.ap()

            with tc.tile_pool(name="gat", bufs=10) as gatp, \
                 tc.tile_pool(name="ssb", bufs=7) as ssbp, \
                 tc.tile_pool(name="psum", bufs=2, space="PSUM") as psump, \
                 tc.tile_pool(name="aggsb", bufs=2) as aggsbp, \
                 tc.tile_pool(name="x0sb", bufs=2) as x0sbp, \
                 tc.tile_pool(name="x0m", bufs=2) as x0mp, \
                 tc.tile_pool(name="xh", bufs=NJ) as xhp, \
                 tc.tile_pool(name="hst", bufs=2) as hstp, \
                 tc.tile_pool(name="cmp", bufs=2) as cmp_, \
                 tc.tile_pool(name="sg", bufs=6) as sgp, \
                 tc.tile_pool(name="gt", bufs=4) as gtp, \
                 tc.tile_pool(name="t1", bufs=3) as t1p, \
                 tc.tile_pool(name="t2", bufs=3) as t2p, \
                 tc.tile_pool(name="tcm", bufs=4) as tcmp:

                def bias_mm(out_ap, lhsT_flat, n, start, stop):
                    nc.tensor.matmul(
                        out=out_ap,
                        lhsT=lhsT_flat.rearrange("o (two m) -> o two m", two=2),
                        rhs=ones8_sb[:, 0:2 * n].rearrange("o (two m) -> o two m", two=2),
                        start=start, stop=stop, perf_mode=DR)

                # persistent LSTM state tiles
                # xh[j]: fp8 [x-ring | h0-ring]: [128, 2, KR, 512]
                xh = [xhp.tile([P, 2 * KR * 512], fp8, tag="xh", name=f"xh_{j}")
                      for j in range(NJ)]
                # h slots: [128, j, slot, 512] bf16 (one big tile per layer)
                hsl = [hstp.tile([P, NJ * 2 * 512], bf16, tag="hs", name=f"hs_{l}")
                       for l in range(2)]
                cm = [cmp_.tile([P, 9 * 512], fp16, tag="cm", name=f"cm_{l}")
                      for l in range(2)]

                def hslot(l, j, t):
                    off = (j * 2 + (t % 2)) * 512
                    return hsl[l][:, off:off + LSTM_TILES[j][1]]

                def xh3(j):
                    return xh[j][:].rearrange("h (p k b) -> h p k b", p=2, k=KR)

                outr = out_d.ap().rearrange("h (t r) -> h t r", t=T)

                # ---- conv (generator yields per ~4-chunk quantum) ----
                def conv_group(d, k0, k1, gs, ss, agg):
                    R = P
                    pp = psump.tile([P, T * H], f32, tag="ps", name=f"pp_{d}_{k0}")
                    for m in range(8):
                        for kk in range(k0, k1 + 1):
                            nc.tensor.matmul(
                                out=pp[:, m * R:(m + 1) * R],
                                lhsT=gs[kk][:, m * P:(m + 1) * P],
                                rhs=ss[kk][:], start=(kk == k0), stop=(kk == k1))
                    if k0 == 0:
                        nc.vector.tensor_copy(agg[:], pp[:, 0:8 * R])
                    else:
                        nc.vector.tensor_tensor(out=agg[:], in0=agg[:],
                                                in1=pp[:, 0:8 * R], op=ALU.add)
                    return pp

                def conv_tail(d, segs, agg, hx):
                    R = P
                    for t in range(T):
                        pb = 64 * (t % 2)
                        for kind, c0, c1 in segs:
                            bias_mm(hx[:, t * R + c0:t * R + c1],
                                    cb8_sb[kind][:, t * 2 * H:(t + 1) * 2 * H],
                                    c1 - c0, start=True, stop=False)
                            nc.tensor.matmul(
                                out=hx[:, t * R + c0:t * R + c1],
                                lhsT=wt_sb[kind][pb:pb + F, t * H:(t + 1) * H],
                                rhs=agg[pb:pb + F, (t // 2) * R + c0:(t // 2) * R + c1],
                                start=False, stop=True)
                    x0t = x0sbp.tile([P, T * R], fp8, tag="x0", name=f"x0t_{d}")
                    if d < int(os.environ.get("PRELU_ACT_D", "33")) and \
                            os.environ.get("NO_PRELU_ACT") != "1":
                        # ramp phase: ACT has idle cycles -> LeakyReLU on ACT
                        # (CoreSim lacks Prelu; the DVE path below is
                        # numerically identical and used for validation)
                        nc.scalar.activation(x0t[:], hx[:, 0:T * R], AF.Prelu,
                                             alpha=0.01)
                    else:
                        # steady state: keep ACT clear. DVE may read only ONE
                        # psum operand: copy to SBUF first.
                        x0m = x0mp.tile([P, T * R], bf16, tag="x0m", name=f"x0m_{d}")
                        nc.vector.tensor_copy(x0m[:], hx[:, 0:T * R])
                        nc.vector.scalar_tensor_tensor(
                            out=x0t[:], in0=x0m[:], scalar=0.01,
                            in1=x0m[:], op0=ALU.mult, op1=ALU.max)
                    j = min(d // 4, NJ - 1)
                    rl = P * (d - 4 * j)

                    def spill(j=j, rl=rl, x0t=x0t):
                        nc.sync.dma_start(
                            x0p[j][:].rearrange("h (t r) -> h t r", t=T)[:, :, rl:rl + P],
                            x0t[:].rearrange("h (t r) -> h t r", t=T))
                    dmaq.append(spill)

                def conv_tile_gen(d):
                    Kd = K[d]
                    agg = aggsbp.tile([P, 8 * P], bf16, tag="agg", name=f"agg_{d}")
                    gs, ss = [], []
                    pp = None
                    for ki in range(Kd):
                        col = int(base[d]) + ki
                        g = gatp.tile([P, T * F], bf16, tag="g", name=f"g_{d}_{ki}")
                        nc.sync.dma_start(g[:], srcdup_ap[col * P:(col + 1) * P, :])
                        s = ssbp.tile([P, P], bf16, tag="s", name=f"s_{d}_{ki}")
                        nc.vector.tensor_scalar(
                            out=s[:], in0=iota_sb[:, 0:P],
                            scalar1=edst_sb[:, col:col + 1], scalar2=ew_sb[:, col:col + 1],
                            op0=ALU.is_equal, op1=ALU.mult)
                        gs.append(g)
                        ss.append(s)
                        if ki % 4 == 3 or ki == Kd - 1:
                            pp = conv_group(d, (ki // 4) * 4, ki, gs, ss, agg)
                            if ki != Kd - 1:
                                yield
                    conv_tail(d, segments[d], agg, pp)
                    yield

                # ---- LSTM unit = (j, t, l) ----
                def front(j, t, l):
                    """gates matmuls + sigmoid + cell-state chain. Returns sg."""
                    B = LSTM_TILES[j][1]
                    gates = psump.tile([P, T * H], f32, tag="ps",
                                       name=f"gates_{l}_{j}_{t}")
                    if l == 0:
                        s = t % KR
                        rhs = xh3(j)[:, 0:2, s:s + 1, 0:B].rearrange(
                            "h p k b -> h (p k) b")
                        for g in range(4):
                            bias_mm(gates[:, g * B:(g + 1) * B],
                                    bias8_sb[0][:, g * 2 * H:(g + 1) * 2 * H], B,
                                    start=True, stop=False)
                            for lvl in range(2):
                                w = w08_sb[:, (g * 2 + lvl) * 256:(g * 2 + lvl + 1) * 256]
                                nc.tensor.matmul(
                                    out=gates[:, g * B:(g + 1) * B],
                                    lhsT=w.rearrange("k (two m) -> k two m", two=2),
                                    rhs=rhs, start=False, stop=(lvl == 1),
                                    perf_mode=DR)
                    else:
                        x_rhs = hslot(0, j, t)
                        h_rhs = hslot(1, j, t - 1) if t > 0 else None
                        for g in range(4):
                            bias_mm(gates[:, g * B:(g + 1) * B],
                                    bias8_sb[1][:, g * 2 * H:(g + 1) * 2 * H], B,
                                    start=True, stop=False)
                            mms = [(whh1_sb, h_rhs)] if h_rhs is not None else []
                            mms.append((wih1_sb, x_rhs))
                            for mi, (w, rhs) in enumerate(mms):
                                nc.tensor.matmul(
                                    out=gates[:, g * B:(g + 1) * B],
                                    lhsT=w[:, g * H:(g + 1) * H],
                                    rhs=rhs, start=False, stop=(mi == len(mms) - 1))
                    sg = sgp.tile([P, 4 * 512], bf16, tag="sg", name=f"sg_{l}_{j}_{t}")
                    nc.scalar.activation(sg[:, 0:4 * B], gates[:, 0:4 * B], AF.Sigmoid)
                    gt = gtp.tile([P, 512], bf16, tag="gt", name=f"gt_{l}_{j}_{t}")
                    nc.vector.tensor_scalar(
                        out=gt[:, 0:B], in0=sg[:, 3 * B:4 * B], scalar1=2.0, scalar2=-1.0,
                        op0=ALU.mult, op1=ALU.add)
                    cs = cm[l][:, LSTM_TILES[j][0]:LSTM_TILES[j][0] + B]
                    if t == 0:
                        nc.vector.tensor_tensor(out=cs, in0=sg[:, 0:B], in1=gt[:, 0:B],
                                                op=ALU.mult)
                    else:
                        t1 = t1p.tile([P, 512], bf16, tag="t1", name=f"t1_{l}_{j}_{t}")
                        nc.vector.tensor_tensor(out=t1[:, 0:B], in0=sg[:, B:2 * B],
                                                in1=cs, op=ALU.mult)
                        t2 = t2p.tile([P, 512], bf16, tag="t2", name=f"t2_{l}_{j}_{t}")
                        nc.vector.tensor_tensor(out=t2[:, 0:B], in0=sg[:, 0:B],
                                                in1=gt[:, 0:B], op=ALU.mult)
                        eng = nc.gpsimd if os.environ.get("CADD_POOL", "0") == "1" \
                            else nc.vector
                        eng.tensor_tensor(out=cs, in0=t1[:, 0:B], in1=t2[:, 0:B],
                                          op=ALU.add)
                    return sg

                def back(j, t, l, sg, tcg, tc0):
                    """h = sig(o)*tanh(c) (+ fp8 ring copy for l0)."""
                    B = LSTM_TILES[j][1]
                    r0 = LSTM_TILES[j][0]
                    hs_ = hslot(l, j, t)
                    nc.vector.tensor_tensor(
                        out=hs_, in0=sg[:, 2 * B:3 * B],
                        in1=tcg[:, r0 - tc0:r0 - tc0 + B], op=ALU.mult)
                    if l == 0 and t + 1 < T:
                        # fp8 copy into the h0-ring for next step's fused DR
                        # (DVE: Pool codegen can't write fp8)
                        s = (t + 1) % KR
                        nc.vector.tensor_copy(
                            xh3(j)[:, 1:2, s:s + 1, 0:B].rearrange(
                                "h a k b -> h (a k b)"),
                            hs_)

                # ---- emission driver ----
                conv_gens = [conv_tile_gen(d) for d in range(N_TILES)]
                conv_i = 0

                def pump_conv():
                    nonlocal conv_i
                    while conv_i < N_TILES:
                        try:
                            next(conv_gens[conv_i])
                            return True
                        except StopIteration:
                            conv_i += 1
                    return False

                # unit schedule: skewed j-streams, layer-interleaved.
                # stream j ready when conv tiles 4j..4j+3 emitted (j=8: all).
                t_next = [[0] * NJ, [0] * NJ]   # per (l, j): next t
                loads = [0] * NJ                # x loads emitted per j
                started = [False] * NJ
                pending = []                    # deferred (tanh+back) closures
                dmaq = []                       # deferred out-store closures

                def j_ready(j):
                    need = N_TILES if j == NJ - 1 else 4 * (j + 1)
                    return conv_i >= need

                def emit_xload(j, t):
                    B = LSTM_TILES[j][1]
                    s = t % KR
                    (nc.gpsimd if os.environ.get("DMA_POOL", "0") == "1"
                     else nc.sync).dma_start(
                        xh3(j)[:, 0:1, s:s + 1, 0:B].rearrange("h a k b -> h (a k b)"),
                        x0p[j][:, t * 512:t * 512 + B])
                    loads[j] = t + 1

                def start_stream(j):
                    B = LSTM_TILES[j][1]
                    # zero h0(-1) ring slot
                    nc.vector.memset(
                        xh3(j)[:, 1:2, 0:1, 0:B].rearrange("h a k b -> h (a k b)"), 0.0)
                    for tt in range(min(KR - 1, T)):
                        emit_xload(j, tt)
                    started[j] = True

                def emit_unit(j, l, t):
                    if l == 0 and t + KR - 1 < T and loads[j] <= t + KR - 1:
                        emit_xload(j, t + KR - 1)
                    sg = front(j, t, l)
                    pending.append((j, t, l, sg, 0))

                def emit_tail(group):
                    # one tanh over the (adjacent-j) cm ranges of the group
                    # (per-stream t may differ: cm ranges are stream-indexed),
                    # then each unit's h-mul
                    l = group[0][2]
                    r0 = min(LSTM_TILES[j][0] for j, _, _, _ in group)
                    cw = sum(LSTM_TILES[j][1] for j, _, _, _ in group)
                    tcg = tcmp.tile([P, 1024], bf16, tag="tc",
                                    name=f"tc_{l}_{group[0][0]}_{group[0][1]}")
                    nc.scalar.activation(tcg[:, 0:cw], cm[l][:, r0:r0 + cw], AF.Tanh)
                    for j, t_, l_, sg in group:
                        back(j, t_, l_, sg, tcg, r0)
                        if l_ == 1:
                            hs_ = hslot(1, j, t_)
                            jr0 = LSTM_TILES[j][0]
                            B = LSTM_TILES[j][1]
                            def store(hs_=hs_, t_=t_, jr0=jr0, B=B):
                                (nc.gpsimd if os.environ.get("DMA_POOL", "0")
                                 == "1" else nc.sync).dma_start(
                                    outr[:, t_:t_ + 1, jr0:jr0 + B].rearrange(
                                        "h t r -> h (t r)"), hs_)
                            dmaq.append(store)

                def flush_pending(drain=False):
                    # emit tanh+back for pending units; batch ADJACENT-j
                    # same-layer units (their t may differ) into one tanh
                    # call. Entries are held at most one extra round.
                    keep = []
                    used = [False] * len(pending)
                    for i, (j, t, l, sg, age) in enumerate(pending):
                        if used[i]:
                            continue
                        partner = None
                        if os.environ.get("PAIR_TANH2", "1") == "1":
                            for i2 in range(len(pending)):
                                if used[i2] or i2 == i:
                                    continue
                                j2, l2 = pending[i2][0], pending[i2][2]
                                if l2 == l and abs(j2 - j) == 1 and \
                                        j2 < NJ - 1 and j < NJ - 1:
                                    partner = i2
                                    break
                        if partner is not None:
                            used[i] = used[partner] = True
                            emit_tail([pending[i][:4], pending[partner][:4]])
                        elif drain or age >= 1 or j == NJ - 1 or len(pending) > 3:
                            used[i] = True
                            emit_tail([pending[i][:4]])
                        else:
                            keep.append((j, t, l, sg, age + 1))
                            used[i] = True
                    pending[:] = keep

                def flush_dma():
                    while dmaq:
                        dmaq.pop(0)()

                # warm-up: conv until stream 0 ready
                while conv_i < 4:
                    pump_conv()

                rr = 0
                guard = 0
                n_started = 0
                last_emit = [-10] * NJ
                while True:
                    guard += 1
                    assert guard < 100000
                    work = pump_conv()
                    if conv_i < 16 and os.environ.get("PUMP3", "0") == "1":
                        work = pump_conv() or work
                        work = pump_conv() or work
                    elif conv_i < N_TILES and n_started < 6:
                        work = pump_conv() or work  # front-load conv in the ramp
                    flush_dma()       # deferred spills/stores (1+ rounds old)
                    flush_pending()   # tanh+back from previous round
                    # start any newly-ready streams
                    for j in range(NJ):
                        if not started[j] and j_ready(j):
                            start_stream(j)
                            n_started += 1
                    # pick the most-behind started stream, with a 3-round
                    # cooldown so a fresh stream's serial chain can't clog
                    # the engine queues; all streams converge and finish
                    # together
                    emitted = False
                    if os.environ.get("SCHED", "rr") == "rr":
                        best = None
                        for k in range(NJ):
                            j = (rr + k) % NJ
                            if started[j] and t_next[0][j] + t_next[1][j] < 2 * T:
                                best = j
                                break
                    else:
                        for cooldown in (3, 0):
                            best, bkey = None, None
                            for k in range(NJ):
                                j = (rr + k) % NJ
                                if not started[j]:
                                    continue
                                prog = t_next[0][j] + t_next[1][j]
                                if prog >= 2 * T:
                                    continue
                                if guard - last_emit[j] < cooldown:
                                    continue
                                if bkey is None or prog < bkey:
                                    best, bkey = j, prog
                            if best is not None:
                                break
                    if best is not None:
                        j = best
                        rr = (j + 1) % NJ
                        last_emit[j] = guard
                        if t_next[0][j] < T and t_next[0][j] <= t_next[1][j]:
                            l, t = 0, t_next[0][j]
                        else:
                            l, t = 1, t_next[1][j]
                        emit_unit(j, l, t)
                        t_next[l][j] = t + 1
                        emitted = True
                    if not emitted:
                        if not work and all(t_next[1][j] >= T for j in range(NJ)):
                            break
                flush_pending(drain=True)
                flush_dma()

    nc.compile()
    return nc


# ----------------------------------------------------------------------------
# Entry points
# ----------------------------------------------------------------------------

def _assemble(results):
    full = np.empty((N_NODE + N_POD + N_SVC, T, H), dtype=np.float32)
    parts_node, parts_pod, parts_svc = [], [], []
    for cidx, res in enumerate(results):
        o = res["out"].astype(np.float32).reshape(H, T, R_CORE).transpose(2, 1, 0)
        n_node = min(NODE_PC, max(0, N_NODE - cidx * NODE_PC))
        n_svc = min(SVC_PC, max(0, N_SVC - cidx * SVC_PC))
        parts_pod.append(o[0:POD_PC])
        svc0 = POD_TILES * P
        parts_svc.append(o[svc0:svc0 + n_svc])
        node0 = (POD_TILES - 1) * P + NODE_ROW0
        parts_node.append(o[node0:node0 + n_node])
    full[0:N_NODE] = np.concatenate(parts_node, axis=0)
    full[N_NODE:N_NODE + N_POD] = np.concatenate(parts_pod, axis=0)
    full[N_NODE + N_POD:] = np.concatenate(parts_svc, axis=0)
    return full


def run(inputs, trace=False):
    from concourse.bass_utils import run_bass_kernel_spmd
    meta, in_maps = _prep(inputs)
    if meta not in _COMPILED:
        _COMPILED[meta] = _build(meta)
    nc = _COMPILED[meta]
    try:
        res = run_bass_kernel_spmd(nc, in_maps, core_ids=list(range(NCORES)), trace=trace)
    except Exception:
        res = run_bass_kernel_spmd(nc, in_maps, core_ids=list(range(NCORES)), trace=trace)
    return _assemble(res.results), res


def kernel(**inputs):
    out, _ = run(inputs, trace=False)
    return out
